# revision 1
# baseline (speedup 1.0000x reference)
"""BoltzmannRouter Trainium2 kernel: 8-core data-parallel Bass implementation.

Full inputs: x (4, 4096, 2048) f32, gate_w (64, 2048) f32.
Output: routing weights (4, 4096, 64) f32 (softmax -> top-44 mask -> renorm).

Sharding: 16384 tokens split 2048/core across 8 NeuronCores; gate weight
replicated. Host pre-transposes each x shard to [D, tokens] so the device
DMA loads contraction-major tiles at full bandwidth, and pre-scales gate_w
by 1/TEMPERATURE (and 2^6 in the fp16 path).

Matmul precision modes (BOLTZ_MM_MODE):
  fp16x3 (default): x and w each split into fp16 high + 2^-12-scaled fp16 low
    parts; scores = 2^-6*(A + 2^-12*B) with A = xh@wh, B = xh@wl + xl@wh
    accumulated in separate PSUM banks. Dropped terms ~2^-22 relative --
    below fp32 PSUM accumulation noise -- at 3 cyc/row instead of fp32's 4.
  fp32: native fp32 matmul (2 half-rate passes per matmul).
"""

import os
import sys

sys.path.insert(0, "/opt/trn_rl_repo")

import numpy as np

D = 2048
E = 64
N_BOTTOM = 20  # 64 experts - 44 active
EPS = 1e-8
NEG_BIG = -1e30
TEMPERATURE = 2.718281828459045
N_CORES = 8
TPC = 2048  # tokens per core
GROUP = 512  # tokens per matmul group (one PSUM bank)

W_SCALE = 64.0  # 2^6: lifts gate_w into fp16-normal range
LO_SCALE = 4096.0  # 2^12: scale on the low fp16 split parts

_MODE = os.environ.get("BOLTZ_MM_MODE", "fp16x3")


def _build_nc():
    import concourse.bacc as bacc
    import concourse.mybir as mybir
    from concourse.masks import make_identity
    from concourse.tile import TileContext

    F32 = mybir.dt.float32
    F16 = mybir.dt.float16
    fp16 = _MODE == "fp16x3"
    mm_dt = F16 if fp16 else getattr(mybir.dt, _MODE, F32)
    kc_n = D // 128
    n_groups = TPC // GROUP
    n_sub = GROUP // 128
    # psum_t carries (-scores) scaled by W_SCALE in the fp16 path
    inv_s = 1.0 / W_SCALE if fp16 else 1.0

    lean_tail = os.environ.get("BOLTZ_LEAN_TAIL", "1") == "1"
    if lean_tail:
        # the stock Tile exit emits drain + barrier + sem-clear + barrier
        # (~8us); the kernel preamble already range-clears the semaphores at
        # the start of every execution, so drain + one barrier suffices
        def _lean_drain_and_barrier(self, tick_clock, wait_clock):
            from concourse.tile import ScopedClock

            drain_inst = self.nc.sync.drain()
            wait_clock.add_sem_waits(
                drain_inst.ins, ScopedClock({None: tick_clock.global_clock})
            )
            self.nc.all_engine_barrier()
            popped = self.nc._tile_sem_poison_stack.pop()
            assert popped is self._sem_poison
            self.sems.allocated()

        TileContext._drain_and_barrier = _lean_drain_and_barrier

    nc = bacc.Bacc(None, target_bir_lowering=False)
    if fp16:
        # xpk[d, g, 0, :] = xh tokens of group g, xpk[d, g, 1, :] = xl
        xpk_d = nc.declare_dram_parameter(
            "xpk", [D, (TPC // GROUP) * 2 * GROUP], F16, isOutput=False
        )
        whl_d = nc.declare_dram_parameter("whl", [D, 2 * E], F16, isOutput=False)
    else:
        xT = nc.declare_dram_parameter("xT", [D, TPC], mm_dt, isOutput=False)
        wT = nc.declare_dram_parameter("wT", [D, E], mm_dt, isOutput=False)
    out = nc.declare_dram_parameter("out", [TPC, E], F32, isOutput=True)

    with TileContext(nc) as tc:
        with (
            tc.tile_pool(name="const", bufs=1) as cpool,
            tc.tile_pool(name="xg", bufs=4) as xpool,
            tc.tile_pool(name="sneg", bufs=2) as spool,
            tc.tile_pool(name="og", bufs=4) as opool,
            tc.tile_pool(name="work", bufs=3) as wkpool,
            tc.tile_pool(name="small", bufs=8) as smpool,
            tc.tile_pool(name="ps_s", bufs=2 if fp16 else 2, space="PSUM") as ps_s_pool,
            tc.tile_pool(name="ps_b", bufs=2, space="PSUM") as ps_b_pool,
            tc.tile_pool(name="ps_t", bufs=4, space="PSUM") as ps_t_pool,
        ):
            ident = cpool.tile([E, E], F32)
            make_identity(nc, ident)
            if fp16:
                # -I/W_SCALE: transposing with a normal matmul by this matrix
                # descales and negates the scores in one shot
                identn = cpool.tile([E, E], F32)
                nc.gpsimd.memset(identn, 0.0)
                nc.gpsimd.affine_select(
                    out=identn,
                    in_=identn,
                    compare_op=mybir.AluOpType.not_equal,
                    fill=-1.0 / W_SCALE,
                    base=0,
                    pattern=[[-1, E]],
                    channel_multiplier=1,
                )

            if fp16:
                whl_sb = cpool.tile([128, kc_n, 2 * E], F16)
                nc.sync.dma_start(
                    out=whl_sb, in_=whl_d[:, :].rearrange("(kc p) e -> p kc e", p=128)
                )
            else:
                w_sb = cpool.tile([128, kc_n, E], mm_dt)
                nc.sync.dma_start(
                    out=w_sb, in_=wT[:, :].rearrange("(kc p) e -> p kc e", p=128)
                )

            og_tiles = []
            for g in range(n_groups):
                tok = slice(g * GROUP, (g + 1) * GROUP)
                if fp16:
                    # per-(group, chunk) tiles so PE pipelines at DMA-arrival
                    # granularity; dispatches spread over 3 queues (SP issue
                    # cost is ~620ns per DMA regardless of size)
                    xhs, xls = [], []
                    gcols = slice(g * 2 * GROUP, (g + 1) * 2 * GROUP)
                    for kc in range(kc_n):
                        row = slice(kc * 128, (kc + 1) * 128)
                        xk = xpool.tile([128, 2 * GROUP], F16, tag=f"x{kc}")
                        nc.sync.dma_start(out=xk, in_=xpk_d[row, gcols])
                        xhs.append(xk[:, :GROUP])
                        xls.append(xk[:, GROUP:])
                    # packed stationary [wh|wl]: one matmul against xh gives
                    # A=wh.T@xh (rows 0:64) and B1=wl.T@xh (rows 64:128); the
                    # second against xl gives B2=wh.T@xl (rows 0:64, the
                    # wl.T@xl block is a free byproduct, never read).
                    # The last parent group is processed in two half-width
                    # passes so the final selection chain drains sooner.
                    snegs = []
                    splits = (
                        [(0, GROUP)]
                        if g < n_groups - 1
                        else [(0, GROUP // 2), (GROUP // 2, GROUP // 2)]
                    )
                    for xoff, w in splits:
                        ps1 = ps_s_pool.tile([2 * E, w], F32, tag="ps_a")
                        ps2 = ps_b_pool.tile([2 * E, w], F32, tag="ps_b")
                        for kc in range(kc_n):
                            nc.tensor.matmul(
                                ps1,
                                lhsT=whl_sb[:, kc, :],
                                rhs=xhs[kc][:, xoff : xoff + w],
                                start=(kc == 0), stop=(kc == kc_n - 1),
                            )
                        for kc in range(kc_n):
                            nc.tensor.matmul(
                                ps2,
                                lhsT=whl_sb[:, kc, :],
                                rhs=xls[kc][:, xoff : xoff + w],
                                start=(kc == 0), stop=(kc == kc_n - 1),
                            )
                        # sneg = A + 2^-12 (B1 + B2) = W_SCALE * scores (sign
                        # and descale are folded into the transpose matrix)
                        b2_sb = spool.tile([E, w], F32, tag="b2_sb")
                        nc.scalar.copy(b2_sb, ps2[:E, :])
                        bs = spool.tile([E, w], F32, tag="bs")
                        nc.vector.tensor_add(bs, ps1[E:, :], b2_sb)
                        sneg = spool.tile([E, w], F32, tag="sneg")
                        nc.vector.scalar_tensor_tensor(
                            out=sneg,
                            in0=bs,
                            scalar=1.0 / LO_SCALE,
                            in1=ps1[:E, :],
                            op0=mybir.AluOpType.mult,
                            op1=mybir.AluOpType.add,
                        )
                        snegs.append((xoff, w, sneg))
                else:
                    xgs = []
                    for kc in range(kc_n):
                        xk = xpool.tile([128, GROUP], mm_dt, tag=f"xg{kc}")
                        nc.sync.dma_start(
                            out=xk, in_=xT[kc * 128 : (kc + 1) * 128, tok]
                        )
                        xgs.append(xk)
                    psum_s = ps_s_pool.tile([E, GROUP], F32, tag="ps_a")
                    for kc in range(kc_n):
                        nc.tensor.matmul(
                            psum_s, lhsT=w_sb[:, kc, :], rhs=xgs[kc],
                            start=(kc == 0), stop=(kc == kc_n - 1),
                        )
                    sneg = spool.tile([E, GROUP], F32, tag="sneg")
                    nc.scalar.mul(sneg, psum_s, -1.0)
                    snegs = [(0, GROUP, sneg)]

                og = opool.tile([128, n_sub, E], F32, tag="og")

                for xoff, w, sneg in snegs:
                  for s in range(w // 128):
                    si = xoff // 128 + s
                    # token-major negated scores [128 tok, 64 e] (x W_SCALE)
                    psum_t = ps_t_pool.tile([128, E], F32, tag="ps_t")
                    if fp16:
                        nc.tensor.matmul(
                            psum_t,
                            lhsT=sneg[:, s * 128 : (s + 1) * 128],
                            rhs=identn,
                        )
                    else:
                        nc.tensor.transpose(
                            psum_t, sneg[:, s * 128 : (s + 1) * 128], ident
                        )

                    # exp bias: -max(scores) = inv_s * min(psum_t)
                    mn = smpool.tile([128, 1], F32, tag="mn")
                    nc.vector.tensor_reduce(
                        mn, psum_t, axis=mybir.AxisListType.X, op=mybir.AluOpType.min
                    )
                    # u = exp(scores - max); S = sum(u)
                    u = wkpool.tile([128, E], F32, tag="u")
                    S = smpool.tile([128, 1], F32, tag="S")
                    nc.scalar.activation(
                        u,
                        psum_t,
                        mybir.ActivationFunctionType.Exp,
                        bias=mn,
                        scale=-1.0,
                        accum_out=S,
                    )

                    # threshold = 21st smallest score (negated domain: top-8
                    # of -scores are the smallest scores; 2x8 removed, then
                    # rank 17-24 -> index 4 = 21st)
                    y = wkpool.tile([128, E], F32, tag="y")
                    nc.vector.tensor_copy(y, psum_t)
                    r1 = smpool.tile([128, 8], F32, tag="r1")
                    nc.vector.max(r1, y)
                    nc.vector.match_replace(y, r1, y, NEG_BIG)
                    r2 = smpool.tile([128, 8], F32, tag="r2")
                    nc.vector.max(r2, y)
                    nc.vector.match_replace(y, r2, y, NEG_BIG)
                    r3 = smpool.tile([128, 8], F32, tag="r3")
                    nc.vector.max(r3, y)
                    thr = r3[:, (N_BOTTOM - 16) : (N_BOTTOM - 16 + 1)]

                    # wm = u * (-scores <= thr); ws = sum(wm)
                    wm = wkpool.tile([128, E], F32, tag="wm")
                    ws = smpool.tile([128, 1], F32, tag="ws")
                    nc.vector.scalar_tensor_tensor(
                        out=wm,
                        in0=psum_t,
                        scalar=thr,
                        in1=u,
                        op0=mybir.AluOpType.is_le,
                        op1=mybir.AluOpType.mult,
                        accum_out=ws,
                    )
                    # den = S*eps + ws; out = wm * (1/den)
                    den = smpool.tile([128, 1], F32, tag="den")
                    nc.vector.scalar_tensor_tensor(
                        out=den,
                        in0=S,
                        scalar=EPS,
                        in1=ws,
                        op0=mybir.AluOpType.mult,
                        op1=mybir.AluOpType.add,
                    )
                    rd = smpool.tile([128, 1], F32, tag="rd")
                    nc.vector.reciprocal(rd, den)
                    nc.vector.tensor_scalar_mul(og[:, si, :], wm, rd)

                og_tiles.append(og)

            # all output DMAs at the very end of the SP stream so no x
            # prefetch dispatch ever queues behind an output wait
            for g, og in enumerate(og_tiles):
                nc.sync.dma_start(
                    out=out[g * GROUP : (g + 1) * GROUP, :].rearrange(
                        "(s p) e -> p s e", p=128
                    ),
                    in_=og,
                )

    nc.finalize()
    return nc


_NC = None
LAST_EXEC_NS = None
LAST_RESULTS = None


def _get_nc():
    global _NC
    if _NC is None:
        _NC = _build_nc()
    return _NC


def _split_fp16(a, scale_hi=1.0):
    """a (f32) -> (hi fp16, lo fp16) with a*scale_hi ~= hi + lo/LO_SCALE."""
    s = (a.astype(np.float32) * np.float32(scale_hi)).astype(np.float32)
    hi = s.astype(np.float16)
    lo = ((s - hi.astype(np.float32)) * np.float32(LO_SCALE)).astype(np.float16)
    return hi, lo


def kernel(x, gate_w, trace=False):
    global LAST_EXEC_NS, LAST_RESULTS
    from concourse.bass_utils import run_bass_kernel_spmd

    x = np.asarray(x)
    gate_w = np.asarray(gate_w)
    Btot = x.shape[0] * x.shape[1]
    x2 = np.ascontiguousarray(x.reshape(Btot, D).astype(np.float32, copy=False))
    wt = np.ascontiguousarray(
        gate_w.astype(np.float32, copy=False).T / np.float32(TEMPERATURE)
    )

    nc = _get_nc()
    in_maps = []
    if _MODE == "fp16x3":
        wh, wl = _split_fp16(wt, W_SCALE)
        whl = np.ascontiguousarray(np.concatenate([wh, wl], axis=1))
        ng = TPC // GROUP
        for i in range(N_CORES):
            shard = np.ascontiguousarray(x2[i * TPC : (i + 1) * TPC].T)
            xh, xl = _split_fp16(shard)
            xpk = np.empty((D, ng, 2, GROUP), np.float16)
            xpk[:, :, 0, :] = xh.reshape(D, ng, GROUP)
            xpk[:, :, 1, :] = xl.reshape(D, ng, GROUP)
            in_maps.append({"xpk": xpk.reshape(D, ng * 2 * GROUP), "whl": whl})
    else:
        for i in range(N_CORES):
            shard = np.ascontiguousarray(x2[i * TPC : (i + 1) * TPC].T)
            in_maps.append({"xT": shard, "wT": wt})

    kwargs = {}
    if trace:
        try:
            import antenv.axon_hooks  # noqa: F401  (shimmed by test harness)

            kwargs["trace"] = True
        except ImportError:
            pass
    res = run_bass_kernel_spmd(nc, in_maps, core_ids=list(range(N_CORES)), **kwargs)
    LAST_EXEC_NS = res.exec_time_ns
    LAST_RESULTS = res
    out = np.concatenate([res.results[i]["out"] for i in range(N_CORES)], axis=0)
    return out.reshape(x.shape[0], x.shape[1], E)



# revision 3
# speedup vs baseline: 1.3065x; 1.3065x over previous
"""BoltzmannRouter Trainium2 kernel: 8-core data-parallel Bass implementation.

Full inputs: x (4, 4096, 2048) f32, gate_w (64, 2048) f32.
Output: routing weights (4, 4096, 64) f32 (softmax -> top-44 mask -> renorm).

Sharding: 16384 tokens split 2048/core across 8 NeuronCores; gate weight
replicated (pre-scaled by 1/TEMPERATURE on host).

The kernel is DMA-bandwidth-bound, so x ships as pure fp16 (half the f32
bytes; adds ~7e-3 rel err vs the 2e-2 gate, dominated by top-44 boundary
swaps between near-tied experts). Host repacks x/w/out DRAM layouts so
every DMA moves >=2KB-contiguous rows per partition (peak packet rate on
the 16 DMA engines) and the device reads land directly in matmul order:

  xg [2048, 2048] f16: row (g*4+c4)*128+p, col cc*512+t holds
     x.T[(c4*4+cc)*128+p, g*512+t] -- one 512KB DMA per (group, c4).
  w2 [128, 1024] f16: w2[p, kc*64+e] = (gate_w.T/TEMP)[kc*128+p, e].
  out [128, 1024] f16: out[p, (g*4+s)*64+e] = weights[g*512+s*128+p, e].

Device pipeline per 512-token group: 16 fp16 matmuls accumulate scores
[64, 512] in PSUM; one Act copy negates+casts to fp16 SBUF; per 128-token
subtile a single fp16 transpose-matmul gives token-major -s, then
Act: u=exp(s); Pool: two fp16 copies of -s; DVE: 3x max8 + 2x
match_replace -> 21st-largest of -s = top-44 threshold, masked weights
with sum accumulator; DVE reciprocal; Act scale -> fp16 out tile.
The softmax max-subtraction and the +1e-8 denominator epsilon are
dropped (|s| <= ~3 so exp cannot overflow; the eps term is <=1e-5
relative -- both far below the fp16 quantization error already accepted).
"""

import os
import sys

sys.path.insert(0, "/opt/trn_rl_repo")

import numpy as np

D = 2048
E = 64
EPS = 1e-8
NEG_BIG = -60000.0  # fp16-representable "removed" marker
TEMPERATURE = 2.718281828459045
N_CORES = 8
TPC = 2048  # tokens per core
GROUP = 512  # tokens per matmul group (one PSUM bank)
N_GROUPS = TPC // GROUP
KC_N = D // 128  # 16 contraction chunks
C4_N = 4  # chunks per x DMA
N_SUB = GROUP // 128


def _build_nc():
    import concourse.bacc as bacc
    import concourse.mybir as mybir
    from concourse.masks import make_identity
    from concourse.tile import TileContext

    F32 = mybir.dt.float32
    F16 = mybir.dt.float16

    lean_tail = os.environ.get("BOLTZ_LEAN_TAIL", "1") == "1"
    if lean_tail:
        # the stock Tile exit emits drain + barrier + sem-clear + barrier
        # (~8us); the kernel preamble already range-clears the semaphores at
        # the start of every execution, so drain + one barrier suffices
        def _lean_drain_and_barrier(self, tick_clock, wait_clock):
            from concourse.tile import ScopedClock

            drain_inst = self.nc.sync.drain()
            wait_clock.add_sem_waits(
                drain_inst.ins, ScopedClock({None: tick_clock.global_clock})
            )
            self.nc.all_engine_barrier()
            popped = self.nc._tile_sem_poison_stack.pop()
            assert popped is self._sem_poison
            self.sems.allocated()

        TileContext._drain_and_barrier = _lean_drain_and_barrier

    nc = bacc.Bacc(None, target_bir_lowering=False)
    xg_d = nc.declare_dram_parameter("xg", [TPC, TPC], F16, isOutput=False)
    w_d = nc.declare_dram_parameter("w2", [128, KC_N * E], F16, isOutput=False)
    out_d = nc.declare_dram_parameter(
        "out", [128, N_GROUPS * N_SUB * E], F16, isOutput=True
    )

    with TileContext(nc) as tc:
        with (
            tc.tile_pool(name="const", bufs=1) as cpool,
            tc.tile_pool(name="xg", bufs=3) as xpool,
            tc.tile_pool(name="s16", bufs=2) as spool,
            tc.tile_pool(name="og", bufs=2) as opool,
            tc.tile_pool(name="work", bufs=4) as wkpool,
            tc.tile_pool(name="small", bufs=8) as smpool,
            tc.tile_pool(name="ps_s", bufs=2, space="PSUM") as ps_s_pool,
            tc.tile_pool(name="ps_t", bufs=4, space="PSUM") as ps_t_pool,
        ):
            ident = cpool.tile([E, E], F16)
            make_identity(nc, ident)

            w_sb = cpool.tile([128, KC_N, E], F16)
            nc.scalar.dma_start(
                out=w_sb, in_=w_d[:, :].rearrange("p (kc e) -> p kc e", kc=KC_N)
            )

            for g in range(N_GROUPS):
                xts = []
                for c4 in range(C4_N):
                    xt = xpool.tile([128, C4_N, GROUP], F16, tag=f"x{c4}")
                    row = (g * C4_N + c4) * 128
                    eng = nc.sync if (g * C4_N + c4) % 2 == 0 else nc.scalar
                    eng.dma_start(
                        out=xt,
                        in_=xg_d[row : row + 128, :].rearrange(
                            "p (cc t) -> p cc t", cc=C4_N
                        ),
                    )
                    xts.append(xt)

                psum_s = ps_s_pool.tile([E, GROUP], F32, tag="ps_s")
                for kc in range(KC_N):
                    nc.tensor.matmul(
                        psum_s,
                        lhsT=w_sb[:, kc, :],
                        rhs=xts[kc // C4_N][:, kc % C4_N, :],
                        start=(kc == 0),
                        stop=(kc == KC_N - 1),
                    )
                # negated scores, fp16, expert-major (feeds transpose lhsT)
                s16 = spool.tile([E, GROUP], F16, tag="s16")
                nc.scalar.mul(s16, psum_s, -1.0)

                og = opool.tile([128, N_SUB, E], F16, tag="og")
                for s in range(N_SUB):
                    # token-major -s [128 tok, 64 e] via fp16 transpose
                    psum_t = ps_t_pool.tile([128, E], F32, tag="ps_t")
                    nc.tensor.matmul(
                        psum_t, lhsT=s16[:, s * 128 : (s + 1) * 128], rhs=ident
                    )

                    # u = exp(s)  (|s| small enough that no max-shift needed)
                    u16 = wkpool.tile([128, E], F16, tag="u16")
                    nc.scalar.activation(
                        u16, psum_t, mybir.ActivationFunctionType.Exp, scale=-1.0
                    )

                    # two fp16 copies of -s: y16 is consumed by the max8
                    # rounds, y2 keeps pre-replacement values for the mask
                    y16 = wkpool.tile([128, E], F16, tag="y16")
                    nc.scalar.copy(y16, psum_t)
                    y2 = wkpool.tile([128, E], F16, tag="y2")
                    nc.gpsimd.tensor_copy(y2, y16)

                    # threshold = 21st largest of -s (= 44th largest score):
                    # two rounds of top-8 removed, then rank 17-24 -> idx 4
                    r1 = smpool.tile([128, 8], F16, tag="r1")
                    nc.vector.max(r1, y16)
                    nc.vector.match_replace(y16, r1, y16, NEG_BIG)
                    r2 = smpool.tile([128, 8], F16, tag="r2")
                    nc.vector.max(r2, y16)
                    nc.vector.match_replace(y16, r2, y16, NEG_BIG)
                    r3 = smpool.tile([128, 8], F16, tag="r3")
                    nc.vector.max(r3, y16)
                    thr = r3[:, 4:5]

                    # wm = u * (-s <= thr); ws = sum(wm)
                    wm = wkpool.tile([128, E], F16, tag="wm")
                    ws = smpool.tile([128, 1], F32, tag="ws")
                    nc.vector.scalar_tensor_tensor(
                        out=wm,
                        in0=y2,
                        scalar=thr,
                        in1=u16,
                        op0=mybir.AluOpType.is_le,
                        op1=mybir.AluOpType.mult,
                        accum_out=ws,
                    )
                    rd = smpool.tile([128, 1], F32, tag="rd")
                    nc.vector.reciprocal(rd, ws)
                    nc.scalar.mul(og[:, s, :], wm, rd)

                nc.gpsimd.dma_start(
                    out=out_d[:, g * N_SUB * E : (g + 1) * N_SUB * E].rearrange(
                        "p (s e) -> p s e", s=N_SUB
                    ),
                    in_=og,
                )

    nc.finalize()
    return nc


_NC = None
LAST_EXEC_NS = None
LAST_RESULTS = None


def _get_nc():
    global _NC
    if _NC is None:
        _NC = _build_nc()
    return _NC


def kernel(x, gate_w, trace=False):
    global LAST_EXEC_NS, LAST_RESULTS
    from concourse.bass_utils import run_bass_kernel_spmd

    x = np.asarray(x)
    gate_w = np.asarray(gate_w)
    Btot = x.shape[0] * x.shape[1]
    x2 = x.reshape(Btot, D)

    # w2[p, kc*64+e] = (gate_w.T / TEMP)[kc*128+p, e]
    wt = (gate_w.astype(np.float32).T / np.float32(TEMPERATURE)).astype(np.float16)
    w2 = np.ascontiguousarray(
        wt.reshape(KC_N, 128, E).transpose(1, 0, 2).reshape(128, KC_N * E)
    )

    nc = _get_nc()
    in_maps = []
    for i in range(N_CORES):
        shard = x2[i * TPC : (i + 1) * TPC].astype(np.float16)
        # xT[kc*128+p, g*512+t] -> xg[(g*4+c4)*128+p, cc*512+t], kc=c4*4+cc
        xT = shard.T.reshape(C4_N, C4_N, 128, N_GROUPS, GROUP)
        xg = np.ascontiguousarray(
            xT.transpose(3, 0, 2, 1, 4).reshape(TPC, TPC)
        )
        in_maps.append({"xg": xg, "w2": w2})

    kwargs = {}
    if trace:
        try:
            import antenv.axon_hooks  # noqa: F401  (shimmed by test harness)

            kwargs["trace"] = True
        except ImportError:
            pass
    res = run_bass_kernel_spmd(nc, in_maps, core_ids=list(range(N_CORES)), **kwargs)
    LAST_EXEC_NS = res.exec_time_ns
    LAST_RESULTS = res
    # out[p, (g*4+s)*64+e] -> weights[g*512+s*128+p, e]
    parts = []
    for i in range(N_CORES):
        o = res.results[i]["out"].reshape(128, N_GROUPS, N_SUB, E)
        parts.append(o.transpose(1, 2, 0, 3).reshape(TPC, E))
    out = np.concatenate(parts, axis=0).astype(np.float32)
    return out.reshape(x.shape[0], x.shape[1], E)


# revision 5
# speedup vs baseline: 1.6315x; 1.2487x over previous
"""BoltzmannRouter Trainium2 kernel: 8-core data-parallel Bass implementation.

Full inputs: x (4, 4096, 2048) f32, gate_w (64, 2048) f32.
Output: routing weights (4, 4096, 64) f32 (softmax -> top-44 mask -> renorm).

Sharding: 16384 tokens split 2048/core across 8 NeuronCores; gate weight
replicated (pre-scaled by 1/TEMPERATURE on host).

The kernel is DMA-bandwidth-bound, so x ships as pure fp16 (half the f32
bytes; adds ~7e-3 rel err vs the 2e-2 gate, dominated by top-44 boundary
swaps between near-tied experts). Host repacks x/w/out DRAM layouts so
every DMA moves >=2KB-contiguous rows per partition (peak packet rate on
the 16 DMA engines) and the device reads land directly in matmul order:

  xg [2048, 2048] f16: row (g*4+c4)*128+p, col cc*512+t holds
     x.T[(c4*4+cc)*128+p, g*512+t] -- one 512KB DMA per (group, c4).
  w2 [128, 1024] f16: w2[p, kc*64+e] = (gate_w.T/TEMP)[kc*128+p, e].
  out [128, 1024] f16: out[p, (g*4+s)*64+e] = weights[g*512+s*128+p, e].

Device pipeline per 512-token group: 16 fp16 matmuls accumulate scores
[64, 512] in PSUM; one Act copy negates+casts to fp16 SBUF; per 128-token
subtile a single fp16 transpose-matmul gives token-major -s, then
Act: u=exp(s); Pool: two fp16 copies of -s; DVE: 3x max8 + 2x
match_replace -> 21st-largest of -s = top-44 threshold, masked weights
with sum accumulator; DVE reciprocal; Act scale -> fp16 out tile.
The softmax max-subtraction and the +1e-8 denominator epsilon are
dropped (|s| <= ~3 so exp cannot overflow; the eps term is <=1e-5
relative -- both far below the fp16 quantization error already accepted).
"""

import os
import sys

sys.path.insert(0, "/opt/trn_rl_repo")

import numpy as np

D = 2048
E = 64
EPS = 1e-8
NEG_BIG = -60000.0  # fp16-representable "removed" marker
TEMPERATURE = 2.718281828459045
N_CORES = 8
TPC = 2048  # tokens per core
GROUP = 512  # tokens per matmul group (one PSUM bank)
N_GROUPS = TPC // GROUP
KC_N = D // 128  # 16 contraction chunks
C4_N = 4  # chunks per x DMA
N_SUB = GROUP // 128


def _build_nc():
    import concourse.bacc as bacc
    import concourse.mybir as mybir
    from concourse.masks import make_identity
    from concourse.tile import TileContext

    F32 = mybir.dt.float32
    F16 = mybir.dt.float16

    lean_tail = os.environ.get("BOLTZ_LEAN_TAIL", "1") == "1"
    if lean_tail:
        # the stock Tile exit emits drain + barrier + sem-clear + barrier
        # (~8us); the kernel preamble already range-clears the semaphores at
        # the start of every execution, so drain + one barrier suffices
        def _lean_drain_and_barrier(self, tick_clock, wait_clock):
            from concourse.tile import ScopedClock

            drain_inst = self.nc.sync.drain()
            wait_clock.add_sem_waits(
                drain_inst.ins, ScopedClock({None: tick_clock.global_clock})
            )
            self.nc.all_engine_barrier()
            popped = self.nc._tile_sem_poison_stack.pop()
            assert popped is self._sem_poison
            self.sems.allocated()

        TileContext._drain_and_barrier = _lean_drain_and_barrier

    nc = bacc.Bacc(None, target_bir_lowering=False)
    xg_d = nc.declare_dram_parameter("xg", [TPC, TPC], F16, isOutput=False)
    w_d = nc.declare_dram_parameter("w2", [128, KC_N * E], F16, isOutput=False)
    out_d = nc.declare_dram_parameter(
        "out", [128, N_GROUPS * N_SUB * E], F16, isOutput=True
    )

    with TileContext(nc) as tc:
        with (
            tc.tile_pool(name="const", bufs=1) as cpool,
            tc.tile_pool(name="xg", bufs=3) as xpool,
            tc.tile_pool(name="s16", bufs=2) as spool,
            tc.tile_pool(name="og", bufs=2) as opool,
            tc.tile_pool(name="work", bufs=4) as wkpool,
            tc.tile_pool(name="small", bufs=8) as smpool,
            tc.tile_pool(name="ps_s", bufs=2, space="PSUM") as ps_s_pool,
            tc.tile_pool(name="ps_t", bufs=4, space="PSUM") as ps_t_pool,
        ):
            ident = cpool.tile([E, E], F16)
            make_identity(nc, ident)

            w_sb = cpool.tile([128, KC_N, E], F16)
            nc.scalar.dma_start(
                out=w_sb, in_=w_d[:, :].rearrange("p (kc e) -> p kc e", kc=KC_N)
            )

            for g in range(N_GROUPS):
                xts = []
                for c4 in range(C4_N):
                    xt = xpool.tile([128, C4_N, GROUP], F16, tag=f"x{c4}")
                    row = (g * C4_N + c4) * 128
                    nc.sync.dma_start(
                        out=xt,
                        in_=xg_d[row : row + 128, :].rearrange(
                            "p (cc t) -> p cc t", cc=C4_N
                        ),
                    )
                    xts.append(xt)

                # the last group runs as two 256-token halves so the final
                # selection chains (the kernel tail) drain sooner
                splits = [(0, GROUP)] if g < N_GROUPS - 1 else [
                    (0, GROUP // 2), (GROUP // 2, GROUP // 2)]
                og = opool.tile([128, N_SUB, E], F16, tag="og")
                for xoff, width in splits:
                    psum_s = ps_s_pool.tile([E, width], F32, tag="ps_s")
                    for kc in range(KC_N):
                        nc.tensor.matmul(
                            psum_s,
                            lhsT=w_sb[:, kc, :],
                            rhs=xts[kc // C4_N][:, kc % C4_N, xoff : xoff + width],
                            start=(kc == 0),
                            stop=(kc == KC_N - 1),
                        )
                    # negated scores, fp16, expert-major (feeds transpose lhsT)
                    s16 = spool.tile([E, width], F16, tag="s16")
                    nc.scalar.mul(s16, psum_s, -1.0)

                    for s in range(width // 128):
                        si = xoff // 128 + s
                        # token-major -s [128 tok, 64 e] via fp16 transpose
                        psum_t = ps_t_pool.tile([128, E], F32, tag="ps_t")
                        nc.tensor.matmul(
                            psum_t, lhsT=s16[:, s * 128 : (s + 1) * 128], rhs=ident
                        )

                        # pristine f32 copy of -s feeds round 1 and the mask
                        y0 = wkpool.tile([128, E], F32, tag="y0")
                        nc.scalar.copy(y0, psum_t)
                        # u = exp(s)  (|s| <= ~3: no max-shift needed)
                        u = wkpool.tile([128, E], F32, tag="u")
                        nc.scalar.activation(
                            u, psum_t, mybir.ActivationFunctionType.Exp, scale=-1.0
                        )

                        # threshold = 21st largest of -s (= 44th largest
                        # score): 2x8 removed into scratch ya, then idx 4 of
                        # round 3 = rank 21
                        r1 = smpool.tile([128, 8], F32, tag="r1")
                        nc.vector.max(r1, y0)
                        ya = wkpool.tile([128, E], F32, tag="ya")
                        nc.vector.match_replace(ya, r1, y0, NEG_BIG)
                        r2 = smpool.tile([128, 8], F32, tag="r2")
                        nc.vector.max(r2, ya)
                        nc.vector.match_replace(ya, r2, ya, NEG_BIG)
                        r3 = smpool.tile([128, 8], F32, tag="r3")
                        nc.vector.max(r3, ya)
                        thr = r3[:, 4:5]

                        # wm = u * (-s <= thr); ws = sum(wm); og = wm / ws
                        wm = wkpool.tile([128, E], F32, tag="wm")
                        ws = smpool.tile([128, 1], F32, tag="ws")
                        nc.vector.scalar_tensor_tensor(
                            out=wm,
                            in0=y0,
                            scalar=thr,
                            in1=u,
                            op0=mybir.AluOpType.is_le,
                            op1=mybir.AluOpType.mult,
                            accum_out=ws,
                        )
                        nc.gpsimd.normalize_recip(og[:, si, :], wm, ws)

                nc.gpsimd.dma_start(
                    out=out_d[:, g * N_SUB * E : (g + 1) * N_SUB * E].rearrange(
                        "p (s e) -> p s e", s=N_SUB
                    ),
                    in_=og,
                )

    nc.finalize()
    return nc


_NC = None
LAST_EXEC_NS = None
LAST_RESULTS = None


def _get_nc():
    global _NC
    if _NC is None:
        _NC = _build_nc()
    return _NC


def kernel(x, gate_w, trace=False):
    global LAST_EXEC_NS, LAST_RESULTS
    from concourse.bass_utils import run_bass_kernel_spmd

    x = np.asarray(x)
    gate_w = np.asarray(gate_w)
    Btot = x.shape[0] * x.shape[1]
    x2 = x.reshape(Btot, D)

    # w2[p, kc*64+e] = (gate_w.T / TEMP)[kc*128+p, e]
    wt = (gate_w.astype(np.float32).T / np.float32(TEMPERATURE)).astype(np.float16)
    w2 = np.ascontiguousarray(
        wt.reshape(KC_N, 128, E).transpose(1, 0, 2).reshape(128, KC_N * E)
    )

    nc = _get_nc()
    in_maps = []
    for i in range(N_CORES):
        shard = x2[i * TPC : (i + 1) * TPC].astype(np.float16)
        # xT[kc*128+p, g*512+t] -> xg[(g*4+c4)*128+p, cc*512+t], kc=c4*4+cc
        xT = shard.T.reshape(C4_N, C4_N, 128, N_GROUPS, GROUP)
        xg = np.ascontiguousarray(
            xT.transpose(3, 0, 2, 1, 4).reshape(TPC, TPC)
        )
        in_maps.append({"xg": xg, "w2": w2})

    kwargs = {}
    if trace:
        try:
            import antenv.axon_hooks  # noqa: F401  (shimmed by test harness)

            kwargs["trace"] = True
        except ImportError:
            pass
    res = run_bass_kernel_spmd(nc, in_maps, core_ids=list(range(N_CORES)), **kwargs)
    LAST_EXEC_NS = res.exec_time_ns
    LAST_RESULTS = res
    # out[p, (g*4+s)*64+e] -> weights[g*512+s*128+p, e]
    parts = []
    for i in range(N_CORES):
        o = res.results[i]["out"].reshape(128, N_GROUPS, N_SUB, E)
        parts.append(o.transpose(1, 2, 0, 3).reshape(TPC, E))
    out = np.concatenate(parts, axis=0).astype(np.float32)
    return out.reshape(x.shape[0], x.shape[1], E)
